# revision 20
# baseline (speedup 1.0000x reference)
"""Bass/Trainium2 kernel for the supervised contrastive loss (triangle v2).

loss = (1/n) * sum_j [ log(colsum_j) - possum_j / (TAL * n_pos_j) ]
with colsum_j = sum_i exp(cos_ij / TAL), possum_j = sum_{i: lab_i=lab_j} cos_ij.

The O(n^2 d) part is colsum; possum collapses to dot(S_{lab_j}, f_j) with
per-class sums S (O(n d)) and is computed on the host, like the row norms.

Symmetry sharding: the Gram matrix is symmetric, so only the upper triangle
of the 16x16 grid of 512-row chunk pairs is computed: 136 pairs instead of
256, a 1.88x PE-work reduction. Core c owns chunk bands A=c and B=15-c and
computes pairs (A, A+d mod 16) for d=0..8 plus (B, B+d mod 16) for d=0..7 —
17 pairs per core, each unordered pair covered exactly once globally.

Per pair (a, b), tiles [128 a-rows, 512 b-cols]:
  - PE: 4 accumulating fp8 DoubleRow matmuls (K=1024) -> PSUM cos tile
  - ACT: exp(cos/TAL) -> et tile (bf16) + accum_out rowsum partial, which is
    the colsum contribution of chunk b to the a-anchors (row sum == col sum
    by symmetry).
  - mirror (b-anchors' contribution from a-rows, a partition-dim sum): DVE
    folds the 4 jt et tiles, then a ones-vector matmul [128,1].T @ fold
    -> [1, 512] PSUM, copied to SBUF. Mirror matmuls are emitted one group
    late so the PE never waits on ACT+DVE.
Self pairs (a == a) need no mirror. The host sums the per-core rowsum/mirror
partials into the full colsum (8k adds, same scale as the baseline's host
epilogue), takes log, and adds the possum/n_pos term.

Head optimizations: per-slot contiguous fp8 feature DMA (host pre-layout),
a dummy exp to pull the ACT table load into the DMA window, and ~6 us of
throwaway ones-matmuls so the PE HAM clock-gate is already warm (2.4 GHz)
when the first real matmul issues.
"""

import numpy as np
import ml_dtypes

import bass_rust
import concourse.bass as bass
import concourse.mybir as mybir
import concourse.tile as tile
from concourse.bass_utils import run_bass_kernel_spmd


def _patch_tile_drain():
    """TRN2 instructions carry at most one semaphore wait, but TileContext's
    exit path attaches every engine/queue wait to a single Drain, which this
    walrus rejects with "Too many sync wait commands". Split the waits across
    single-wait NoOps ahead of the drain instead."""
    if getattr(tile.TileContext, "_drain_waits_split", False):
        return

    def _drain_and_barrier(self, tick_clock, wait_clock):
        probe = self.nc.sync.nop()
        wait_clock.add_sem_waits(
            probe.ins, bass_rust.ScopedClock({None: tick_clock.global_clock})
        )
        si = probe.ins.sync_info
        waits = list(si.on_wait) if si is not None else []
        if len(waits) > 1:
            probe.ins.sync_info = bass_rust.SyncInfo(
                on_wait=[waits[0]], on_update=list(si.on_update)
            )
            for w in waits[1:]:
                extra = self.nc.sync.nop()
                extra.ins.sync_info = bass_rust.SyncInfo(on_wait=[w], on_update=[])
        self.nc.sync.drain()
        self.nc.all_engine_barrier()
        assert self.sems is not None
        popped = self.nc._tile_sem_poison_stack.pop()
        assert popped is self._sem_poison
        self.nc.clear_and_free_semaphores(list(self.sems.allocated().values()))
        self.nc.all_engine_barrier()

    tile.TileContext._drain_and_barrier = _drain_and_barrier
    tile.TileContext._drain_waits_split = True


_patch_tile_drain()


def _patch_split_multiwait():
    """This container's walrus accepts only ONE semaphore wait per TPB
    instruction (setupSyncWait: "Too many sync wait commands"), but Tile's
    add_semaphores pass attaches up to 3. Rewrite the BIR before compiling:
    move all but the last wait of each instruction onto single-wait NoOps
    inserted just before it on the same engine (same AND-of-waits semantics,
    engine programs execute in order)."""
    import orjson
    import concourse.bass_utils as _bu
    import concourse.bass2jax as _b2j

    if getattr(_bu, "_multiwait_split_installed", False):
        return
    orig = _bu.compile_bir_kernel

    def compile_bir_kernel(bir_json, tmpdir, neff_name="file.neff"):
        bir = orjson.loads(bir_json)
        changed = False
        for fn in bir.get("functions", []):
            for bb in fn.get("blocks", []):
                out = []
                for ins in bb.get("instructions", []):
                    si = ins.get("sync_info")
                    w = si.get("on_wait", []) if si else []
                    if len(w) > 1:
                        changed = True
                        for j, extra in enumerate(w[:-1]):
                            out.append(
                                {
                                    "debug": ins.get("debug", 0),
                                    "engine": ins["engine"],
                                    "ins": [],
                                    "outs": [],
                                    "name": f"{ins['name']}-sw{j}",
                                    "opcode": "NoOp",
                                    "sync_info": {"on_update": [], "on_wait": [extra]},
                                }
                            )
                        si["on_wait"] = [w[-1]]
                    out.append(ins)
                bb["instructions"] = out
        if changed:
            bir_json = orjson.dumps(bir)
        return orig(bir_json, tmpdir, neff_name=neff_name)

    _bu.compile_bir_kernel = compile_bir_kernel
    _b2j.compile_bir_kernel = compile_bir_kernel
    _bu._multiwait_split_installed = True


_patch_split_multiwait()

N = 8192          # rows (Gram dimension)
D = 1024          # feature dim (contraction)
P = 128           # partitions
NCORES = 8
NCH = 16          # 512-row chunks
CW = 512          # chunk width
KT = D // P       # k subtiles (8)
NSLOT = 17        # rhs chunk slots per core (9 band-A + 8 band-B)
NJT = 4           # 128-row j-tiles per chunk
TAL = 0.07
NWARM = 4         # throwaway PE warm-up matmuls during the DMA head

BF16 = mybir.dt.bfloat16
F32 = mybir.dt.float32
FP8 = mybir.dt.float8e4
NP_FP8 = ml_dtypes.float8_e4m3

# Static per-core group schedule (bands at slots 0 and 9; identical on every
# core — per-core chunk identity comes from the host-side slot layout).
# Each group: (band, gi, [rhs slot ks]). Ordering: the self pair first (compute
# starts after one slot DMA), early groups kept single so they don't outrun the
# slot DMA stream, and the band-B self pair last — it has no mirror, so the PE
# tail never waits on an ACT+DVE fold.
GROUPS = [
    (0, 0, [0]), (0, 1, [2, 3]), (0, 2, [4, 5]), (0, 3, [6, 7]), (0, 4, [1, 8]),
    (1, 1, [10]), (1, 2, [11, 12]), (1, 3, [13, 14]), (1, 4, [15, 16]),
    (1, 0, [9]),
]
NGI = 5           # accum slots per band
NMIR = 15         # non-self pairs per core

_CACHE: dict = {}

# test.py introspection: last BassKernelResults from run_bass_kernel_spmd
LAST_RESULTS = None


def _build_bass() -> bass.Bass:
    nc = bass.Bass(trn_type="TRN2")

    feaC = nc.dram_tensor("feaC", [P, NSLOT, KT, CW], FP8, kind="ExternalInput")
    rows_out = nc.dram_tensor("rows_out", [P, 8, NGI], F32, kind="ExternalOutput")
    mir_out = nc.dram_tensor("mir_out", [1, NMIR * CW], F32, kind="ExternalOutput")

    with tile.TileContext(nc) as tc:
        with (
            tc.tile_pool(name="singles", bufs=1) as singles,
            tc.tile_pool(name="et", bufs=12) as et_pool,
            tc.tile_pool(name="fold", bufs=3) as fold_pool,
            tc.tile_pool(name="psum", bufs=3, space="PSUM") as psum_pool,
            tc.tile_pool(name="mpsum", bufs=2, space="PSUM") as mpsum_pool,
        ):
            # Scratch operand for warm-up work: filled by a cheap on-chip
            # memset so the PE warm-up and ACT table load start right after
            # the preamble with no DMA dependency.
            scratch = singles.tile([P, CW], BF16)
            nc.vector.memset(scratch[:], 1.0)
            etw = et_pool.tile([P, 4], BF16, tag="warm_act")
            nc.scalar.activation(
                out=etw[:], in_=scratch[:, 0:4],
                func=mybir.ActivationFunctionType.Exp, scale=1.0 / TAL,
            )
            # PE HAM warm-up: throwaway matmuls keep the PE busy (and the
            # clock gate opening) while the first feature slot streams in.
            for _ in range(NWARM):
                wp = mpsum_pool.tile([1, CW], F32, tag="m")
                nc.tensor.matmul(
                    wp[:], scratch[:, 0:1], scratch[:, :], start=True, stop=True
                )

            ones_t = singles.tile([P, 1], BF16)
            nc.vector.memset(ones_t[:], 1.0)

            slots = singles.tile([P, NSLOT, KT, CW], FP8)
            # Tiny dummy transfers first: absorb DMA queue spin-up cost so
            # slot 0 streams at full rate. Slot 0 is split into K-halves
            # across the two HWDGE queues (Sync + Scalar), halving its
            # latency — the queues share HBM bandwidth, so later slots stay
            # serial on Sync where arrival order matches consumption order.
            dmaw = singles.tile([P, 64], FP8)
            nc.sync.dma_start(out=dmaw[:, 0:32], in_=feaC[:, 0, 0, 0:32])
            nc.scalar.dma_start(out=dmaw[:, 32:64], in_=feaC[:, 0, 0, 32:64])
            nc.sync.dma_start(out=slots[:, 0, 0:4], in_=feaC[:, 0, 0:4])
            nc.scalar.dma_start(out=slots[:, 0, 4:8], in_=feaC[:, 0, 4:8])
            for s in range(1, NSLOT):
                nc.sync.dma_start(out=slots[:, s], in_=feaC[:, s])

            rows_parts = singles.tile([P, 8, NGI], F32)
            # band B fills only gi 0..4; zero so the epilogue reduce over all
            # NGI columns is correct
            nc.vector.memset(rows_parts[:], 0.0)
            mirror_sb = singles.tile([1, NMIR * CW], F32)

            mcol = 0
            pending = []  # deferred mirror matmuls: (fold_tile, [ks])

            def flush_mirrors():
                nonlocal mcol
                for F3, ks in pending:
                    for h, _k in enumerate(ks):
                        mp = mpsum_pool.tile([1, CW], F32, tag="m")
                        nc.tensor.matmul(
                            mp[:], ones_t[:, 0:1], F3[:, h * CW:(h + 1) * CW],
                            start=True, stop=True,
                        )
                        nc.vector.tensor_copy(
                            mirror_sb[0:1, mcol * CW:(mcol + 1) * CW], mp[:]
                        )
                        mcol += 1
                pending.clear()

            for band, gi, ks in GROUPS:
                L = 0 if band == 0 else 9
                W = len(ks) * CW
                ets = []
                for jt in range(NJT):
                    ps = psum_pool.tile([P, 2 * CW], F32, tag="ps")
                    for gsi, k in enumerate(ks):
                        psh = ps[:, gsi * CW:(gsi + 1) * CW]
                        for k2 in range(KT // 2):
                            nc.tensor.matmul(
                                psh,
                                slots[:, L, 2 * k2:2 * k2 + 2, jt * P:(jt + 1) * P],
                                slots[:, k, 2 * k2:2 * k2 + 2, :],
                                start=(k2 == 0),
                                stop=(k2 == KT // 2 - 1),
                                perf_mode=mybir.MatmulPerfMode.DoubleRow,
                            )
                    et = et_pool.tile([P, 2 * CW], BF16, tag="et")
                    nc.scalar.activation(
                        out=et[:, :W],
                        in_=ps[:, :W],
                        func=mybir.ActivationFunctionType.Exp,
                        scale=1.0 / TAL,
                        accum_out=rows_parts[:, band * 4 + jt, gi:gi + 1],
                    )
                    ets.append(et)
                # mirror fold for non-self groups (self pair k == L needs none)
                if ks != [L]:
                    t01 = fold_pool.tile([P, 2 * CW], BF16, tag="tmpA")
                    nc.vector.tensor_add(t01[:, :W], ets[0][:, :W], ets[1][:, :W])
                    t23 = fold_pool.tile([P, 2 * CW], BF16, tag="tmpB")
                    nc.vector.tensor_add(t23[:, :W], ets[2][:, :W], ets[3][:, :W])
                    F3 = fold_pool.tile([P, 2 * CW], BF16, tag="F3")
                    nc.vector.tensor_add(F3[:, :W], t01[:, :W], t23[:, :W])
                    pending.append((F3, ks))
                else:
                    flush_mirrors()
                # two-group deferral: emit a group's mirrors two groups of
                # matmuls later, so the PE never waits on its ACT+DVE fold
                # (single-pair groups only provide ~4 us of cover, less than
                # the ~4 us fold latency, so one group is not enough)
                if len(pending) > 2:
                    done, pending = pending[:-2], pending[-2:]
                    for F3, ks2 in done:
                        for h, _k in enumerate(ks2):
                            mp = mpsum_pool.tile([1, CW], F32, tag="m")
                            nc.tensor.matmul(
                                mp[:], ones_t[:, 0:1], F3[:, h * CW:(h + 1) * CW],
                                start=True, stop=True,
                            )
                            nc.vector.tensor_copy(
                                mirror_sb[0:1, mcol * CW:(mcol + 1) * CW], mp[:]
                            )
                            mcol += 1
            flush_mirrors()
            assert mcol == NMIR, mcol

            # raw partials out (host folds the gi axis); parallel queues so
            # the two triggers don't serialize in the tail
            nc.scalar.dma_start(out=rows_out[:, :, :], in_=rows_parts[:])
            nc.sync.dma_start(out=mir_out[:, :], in_=mirror_sb[:])

    return nc


def _slot_chunks(c: int) -> list[int]:
    A, B = c, 15 - c
    return [(A + d) % NCH for d in range(9)] + [(B + d) % NCH for d in range(8)]


def _prep_inputs(feature: np.ndarray):
    fea = np.asarray(feature, dtype=np.float32)
    norms = np.sqrt((fea.astype(np.float64) ** 2).sum(axis=1)).astype(np.float32)
    fean = fea / norms[:, None]
    fean8 = fean.astype(NP_FP8)
    # [P, KT, N]: partition-major fp8 features, k-subtile layout matching the
    # DoubleRow matmul APs ((k*128+p, i) -> [p, k, i])
    At = np.ascontiguousarray(fean8.T.reshape(KT, P, N).transpose(1, 0, 2))
    in_maps = []
    for c in range(NCORES):
        chunks = _slot_chunks(c)
        feaC = np.ascontiguousarray(
            np.stack([At[:, :, g * CW:(g + 1) * CW] for g in chunks], axis=1)
        )
        in_maps.append({"feaC": feaC})
    return fean, in_maps


def _mirror_chunks(c: int) -> list[int]:
    """Global chunk index that each mirror column block belongs to, in device
    emission (mcol) order: groups in GROUPS order, non-self ks in order."""
    chunks = _slot_chunks(c)
    out = []
    for band, _gi, ks in GROUPS:
        L = 0 if band == 0 else 9
        if ks == [L]:
            continue
        for k in ks:
            out.append(chunks[k])
    return out


def kernel(feature: np.ndarray, label: np.ndarray) -> np.ndarray:
    global LAST_RESULTS
    if "nc" not in _CACHE:
        _CACHE["nc"] = _build_bass()
    nc = _CACHE["nc"]
    fean, in_maps = _prep_inputs(feature)
    res = run_bass_kernel_spmd(nc, in_maps, core_ids=list(range(NCORES)))
    LAST_RESULTS = res

    colsum = np.zeros(N, dtype=np.float64)
    for c in range(NCORES):
        r = res.results[c]
        rows = r["rows_out"].astype(np.float64).sum(axis=2)   # [P, 8, NGI] -> [P, 8]
        mir = r["mir_out"].reshape(NMIR, CW).astype(np.float64)
        A, B = c, 15 - c
        for band, g in ((0, A), (1, B)):
            for jt in range(NJT):
                colsum[g * CW + jt * P: g * CW + (jt + 1) * P] += rows[:, band * 4 + jt]
        for idx, g2 in enumerate(_mirror_chunks(c)):
            colsum[g2 * CW:(g2 + 1) * CW] += mir[idx]

    lab = np.asarray(label)
    counts = np.bincount(lab, minlength=int(lab.max()) + 1)
    order = np.argsort(lab, kind="stable")
    sorted_lab = lab[order]
    starts = np.concatenate(([0], np.nonzero(np.diff(sorted_lab))[0] + 1))
    S = np.zeros((counts.size, D), dtype=np.float32)
    S[sorted_lab[starts]] = np.add.reduceat(fean[order], starts, axis=0)
    possum = np.einsum("ij,ij->i", fean, S[lab]).astype(np.float64)

    loss_j = np.log(colsum) - possum / (TAL * counts[lab])
    return np.float32(loss_j.sum() / N)


# revision 21
# speedup vs baseline: 1.1850x; 1.1850x over previous
"""Bass/Trainium2 kernel for the supervised contrastive loss (triangle v2).

loss = (1/n) * sum_j [ log(colsum_j) - possum_j / (TAL * n_pos_j) ]
with colsum_j = sum_i exp(cos_ij / TAL), possum_j = sum_{i: lab_i=lab_j} cos_ij.

The O(n^2 d) part is colsum; possum collapses to dot(S_{lab_j}, f_j) with
per-class sums S (O(n d)) and is computed on the host, like the row norms.

Symmetry sharding: the Gram matrix is symmetric, so only the upper triangle
of the 16x16 grid of 512-row chunk pairs is computed: 136 pairs instead of
256, a 1.88x PE-work reduction. Core c owns chunk bands A=c and B=15-c and
computes pairs (A, A+d mod 16) for d=0..8 plus (B, B+d mod 16) for d=0..7 —
17 pairs per core, each unordered pair covered exactly once globally.

Per pair (a, b), tiles [128 a-rows, 512 b-cols]:
  - PE: 4 accumulating fp8 DoubleRow matmuls (K=1024) -> PSUM cos tile
  - ACT: exp(cos/TAL) -> et tile (bf16) + accum_out rowsum partial, which is
    the colsum contribution of chunk b to the a-anchors (row sum == col sum
    by symmetry).
  - mirror (b-anchors' contribution from a-rows, a partition-dim sum): DVE
    folds the 4 jt et tiles, then a ones-vector matmul [128,1].T @ fold
    -> [1, 512] PSUM, copied to SBUF. Mirror matmuls are emitted one group
    late so the PE never waits on ACT+DVE.
Self pairs (a == a) need no mirror. The host sums the per-core rowsum/mirror
partials into the full colsum (8k adds, same scale as the baseline's host
epilogue), takes log, and adds the possum/n_pos term.

Head optimizations: per-slot contiguous fp8 feature DMA (host pre-layout),
a dummy exp to pull the ACT table load into the DMA window, and ~6 us of
throwaway ones-matmuls so the PE HAM clock-gate is already warm (2.4 GHz)
when the first real matmul issues.
"""

import numpy as np
import ml_dtypes

import bass_rust
import concourse.bass as bass
import concourse.mybir as mybir
import concourse.tile as tile
from concourse.bass_utils import run_bass_kernel_spmd


def _patch_tile_drain():
    """TRN2 instructions carry at most one semaphore wait, but TileContext's
    exit path attaches every engine/queue wait to a single Drain, which this
    walrus rejects with "Too many sync wait commands". Split the waits across
    single-wait NoOps ahead of the drain instead."""
    if getattr(tile.TileContext, "_drain_waits_split", False):
        return

    def _drain_and_barrier(self, tick_clock, wait_clock):
        probe = self.nc.sync.nop()
        wait_clock.add_sem_waits(
            probe.ins, bass_rust.ScopedClock({None: tick_clock.global_clock})
        )
        si = probe.ins.sync_info
        waits = list(si.on_wait) if si is not None else []
        if len(waits) > 1:
            probe.ins.sync_info = bass_rust.SyncInfo(
                on_wait=[waits[0]], on_update=list(si.on_update)
            )
            for w in waits[1:]:
                extra = self.nc.sync.nop()
                extra.ins.sync_info = bass_rust.SyncInfo(on_wait=[w], on_update=[])
        self.nc.sync.drain()
        self.nc.all_engine_barrier()
        assert self.sems is not None
        popped = self.nc._tile_sem_poison_stack.pop()
        assert popped is self._sem_poison
        self.nc.clear_and_free_semaphores(list(self.sems.allocated().values()))
        self.nc.all_engine_barrier()

    tile.TileContext._drain_and_barrier = _drain_and_barrier
    tile.TileContext._drain_waits_split = True


_patch_tile_drain()


def _patch_split_multiwait():
    """This container's walrus accepts only ONE semaphore wait per TPB
    instruction (setupSyncWait: "Too many sync wait commands"), but Tile's
    add_semaphores pass attaches up to 3. Rewrite the BIR before compiling:
    move all but the last wait of each instruction onto single-wait NoOps
    inserted just before it on the same engine (same AND-of-waits semantics,
    engine programs execute in order)."""
    import orjson
    import concourse.bass_utils as _bu
    import concourse.bass2jax as _b2j

    if getattr(_bu, "_multiwait_split_installed", False):
        return
    orig = _bu.compile_bir_kernel

    def compile_bir_kernel(bir_json, tmpdir, neff_name="file.neff"):
        bir = orjson.loads(bir_json)
        changed = False
        for fn in bir.get("functions", []):
            for bb in fn.get("blocks", []):
                out = []
                for ins in bb.get("instructions", []):
                    si = ins.get("sync_info")
                    w = si.get("on_wait", []) if si else []
                    if len(w) > 1:
                        changed = True
                        for j, extra in enumerate(w[:-1]):
                            out.append(
                                {
                                    "debug": ins.get("debug", 0),
                                    "engine": ins["engine"],
                                    "ins": [],
                                    "outs": [],
                                    "name": f"{ins['name']}-sw{j}",
                                    "opcode": "NoOp",
                                    "sync_info": {"on_update": [], "on_wait": [extra]},
                                }
                            )
                        si["on_wait"] = [w[-1]]
                    out.append(ins)
                bb["instructions"] = out
        if changed:
            bir_json = orjson.dumps(bir)
        return orig(bir_json, tmpdir, neff_name=neff_name)

    _bu.compile_bir_kernel = compile_bir_kernel
    _b2j.compile_bir_kernel = compile_bir_kernel
    _bu._multiwait_split_installed = True


_patch_split_multiwait()

N = 8192          # rows (Gram dimension)
D = 1024          # feature dim (contraction)
P = 128           # partitions
NCORES = 8
NCH = 16          # 512-row chunks
CW = 512          # chunk width
KT = D // P       # k subtiles (8)
NSLOT = 17        # rhs chunk slots per core (9 band-A + 8 band-B)
NJT = 4           # 128-row j-tiles per chunk
TAL = 0.07
NWARM = 6         # throwaway PE warm-up matmuls during the DMA head

BF16 = mybir.dt.bfloat16
F32 = mybir.dt.float32
FP8 = mybir.dt.float8e4
NP_FP8 = ml_dtypes.float8_e4m3

# Static per-core group schedule (bands at slots 0 and 9; identical on every
# core — per-core chunk identity comes from the host-side slot layout).
# Each group: (band, gi, [rhs slot ks]). Ordering: the self pair first (compute
# starts after one slot DMA), early groups kept single so they don't outrun the
# slot DMA stream, and the band-B self pair last — it has no mirror, so the PE
# tail never waits on an ACT+DVE fold.
GROUPS = [
    (0, 0, [0]), (0, 1, [2, 3]), (0, 2, [4, 5]), (0, 3, [6, 7]), (0, 4, [1, 8]),
    (1, 1, [10]), (1, 2, [11, 12]), (1, 3, [13, 14]), (1, 4, [15, 16]),
    (1, 0, [9]),
]
NGI = 5           # accum slots per band
NMIR = 15         # non-self pairs per core

_CACHE: dict = {}

# test.py introspection: last BassKernelResults from run_bass_kernel_spmd
LAST_RESULTS = None


def _build_bass() -> bass.Bass:
    nc = bass.Bass(trn_type="TRN2")

    feaC = nc.dram_tensor("feaC", [P, NSLOT, KT, CW], FP8, kind="ExternalInput")
    rows_out = nc.dram_tensor("rows_out", [P, 8, NGI], F32, kind="ExternalOutput")
    mir_out = nc.dram_tensor("mir_out", [1, NMIR * CW], F32, kind="ExternalOutput")

    with tile.TileContext(nc) as tc:
        with (
            tc.tile_pool(name="singles", bufs=1) as singles,
            tc.tile_pool(name="et", bufs=12) as et_pool,
            tc.tile_pool(name="fold", bufs=3) as fold_pool,
            tc.tile_pool(name="psum", bufs=3, space="PSUM") as psum_pool,
            tc.tile_pool(name="mpsum", bufs=2, space="PSUM") as mpsum_pool,
        ):
            # Scratch operand for warm-up work: filled by a cheap on-chip
            # memset so the PE warm-up and ACT table load start right after
            # the preamble with no DMA dependency.
            scratch = singles.tile([P, CW], BF16)
            nc.vector.memset(scratch[:], 1.0)
            etw = et_pool.tile([P, 4], BF16, tag="warm_act")
            nc.scalar.activation(
                out=etw[:], in_=scratch[:, 0:4],
                func=mybir.ActivationFunctionType.Exp, scale=1.0 / TAL,
            )
            # PE HAM warm-up: throwaway matmuls keep the PE busy (and the
            # clock gate opening) while the first feature slot streams in.
            for _ in range(NWARM):
                wp = mpsum_pool.tile([1, CW], F32, tag="m")
                nc.tensor.matmul(
                    wp[:], scratch[:, 0:1], scratch[:, :], start=True, stop=True
                )

            ones_t = singles.tile([P, 1], BF16)
            nc.vector.memset(ones_t[:], 1.0)

            slots = singles.tile([P, NSLOT, KT, CW], FP8)
            # Tiny dummy transfers first: absorb DMA queue spin-up cost so
            # slot 0 streams at full rate. Slot 0 is split into K-halves
            # across the two HWDGE queues (Sync + Scalar), halving its
            # latency — the queues share HBM bandwidth, so later slots stay
            # serial on Sync where arrival order matches consumption order.
            dmaw = singles.tile([P, 64], FP8)
            nc.sync.dma_start(out=dmaw[:, 0:32], in_=feaC[:, 0, 0, 0:32])
            nc.scalar.dma_start(out=dmaw[:, 32:64], in_=feaC[:, 0, 0, 32:64])
            nc.sync.dma_start(out=slots[:, 0, 0:4], in_=feaC[:, 0, 0:4])
            nc.scalar.dma_start(out=slots[:, 0, 4:8], in_=feaC[:, 0, 4:8])
            for s in range(1, NSLOT):
                nc.sync.dma_start(out=slots[:, s], in_=feaC[:, s])

            rows_parts = singles.tile([P, 8, NGI], F32)
            # band B fills only gi 0..4; zero so the epilogue reduce over all
            # NGI columns is correct
            nc.vector.memset(rows_parts[:], 0.0)
            mirror_sb = singles.tile([1, NMIR * CW], F32)

            mcol = 0
            pending = []  # deferred mirror matmuls: (fold_tile, [ks])

            def flush_mirrors():
                nonlocal mcol
                for F3, ks in pending:
                    for h, _k in enumerate(ks):
                        mp = mpsum_pool.tile([1, CW], F32, tag="m")
                        nc.tensor.matmul(
                            mp[:], ones_t[:, 0:1], F3[:, h * CW:(h + 1) * CW],
                            start=True, stop=True,
                        )
                        nc.vector.tensor_copy(
                            mirror_sb[0:1, mcol * CW:(mcol + 1) * CW], mp[:]
                        )
                        mcol += 1
                pending.clear()

            for band, gi, ks in GROUPS:
                L = 0 if band == 0 else 9
                W = len(ks) * CW
                ets = []
                for jt in range(NJT):
                    ps = psum_pool.tile([P, 2 * CW], F32, tag="ps")
                    for gsi, k in enumerate(ks):
                        psh = ps[:, gsi * CW:(gsi + 1) * CW]
                        for k2 in range(KT // 2):
                            nc.tensor.matmul(
                                psh,
                                slots[:, L, 2 * k2:2 * k2 + 2, jt * P:(jt + 1) * P],
                                slots[:, k, 2 * k2:2 * k2 + 2, :],
                                start=(k2 == 0),
                                stop=(k2 == KT // 2 - 1),
                                perf_mode=mybir.MatmulPerfMode.DoubleRow,
                            )
                    et = et_pool.tile([P, 2 * CW], BF16, tag="et")
                    nc.scalar.activation(
                        out=et[:, :W],
                        in_=ps[:, :W],
                        func=mybir.ActivationFunctionType.Exp,
                        scale=1.0 / TAL,
                        accum_out=rows_parts[:, band * 4 + jt, gi:gi + 1],
                    )
                    ets.append(et)
                # mirror fold for non-self groups (self pair k == L needs none)
                if ks != [L]:
                    t01 = fold_pool.tile([P, 2 * CW], BF16, tag="tmpA")
                    nc.vector.tensor_add(t01[:, :W], ets[0][:, :W], ets[1][:, :W])
                    t23 = fold_pool.tile([P, 2 * CW], BF16, tag="tmpB")
                    nc.vector.tensor_add(t23[:, :W], ets[2][:, :W], ets[3][:, :W])
                    F3 = fold_pool.tile([P, 2 * CW], BF16, tag="F3")
                    nc.vector.tensor_add(F3[:, :W], t01[:, :W], t23[:, :W])
                    pending.append((F3, ks))
                else:
                    flush_mirrors()
                # two-group deferral: emit a group's mirrors two groups of
                # matmuls later, so the PE never waits on its ACT+DVE fold
                # (single-pair groups only provide ~4 us of cover, less than
                # the ~4 us fold latency, so one group is not enough)
                if len(pending) > 2:
                    done, pending = pending[:-2], pending[-2:]
                    for F3, ks2 in done:
                        for h, _k in enumerate(ks2):
                            mp = mpsum_pool.tile([1, CW], F32, tag="m")
                            nc.tensor.matmul(
                                mp[:], ones_t[:, 0:1], F3[:, h * CW:(h + 1) * CW],
                                start=True, stop=True,
                            )
                            nc.vector.tensor_copy(
                                mirror_sb[0:1, mcol * CW:(mcol + 1) * CW], mp[:]
                            )
                            mcol += 1
            flush_mirrors()
            assert mcol == NMIR, mcol

            # raw partials out (host folds the gi axis); parallel queues so
            # the two triggers don't serialize in the tail
            nc.scalar.dma_start(out=rows_out[:, :, :], in_=rows_parts[:])
            nc.sync.dma_start(out=mir_out[:, :], in_=mirror_sb[:])

    return nc


def _slot_chunks(c: int) -> list[int]:
    A, B = c, 15 - c
    return [(A + d) % NCH for d in range(9)] + [(B + d) % NCH for d in range(8)]


def _prep_inputs(feature: np.ndarray):
    fea = np.asarray(feature, dtype=np.float32)
    norms = np.sqrt((fea.astype(np.float64) ** 2).sum(axis=1)).astype(np.float32)
    fean = fea / norms[:, None]
    fean8 = fean.astype(NP_FP8)
    # [P, KT, N]: partition-major fp8 features, k-subtile layout matching the
    # DoubleRow matmul APs ((k*128+p, i) -> [p, k, i])
    At = np.ascontiguousarray(fean8.T.reshape(KT, P, N).transpose(1, 0, 2))
    in_maps = []
    for c in range(NCORES):
        chunks = _slot_chunks(c)
        feaC = np.ascontiguousarray(
            np.stack([At[:, :, g * CW:(g + 1) * CW] for g in chunks], axis=1)
        )
        in_maps.append({"feaC": feaC})
    return fean, in_maps


def _mirror_chunks(c: int) -> list[int]:
    """Global chunk index that each mirror column block belongs to, in device
    emission (mcol) order: groups in GROUPS order, non-self ks in order."""
    chunks = _slot_chunks(c)
    out = []
    for band, _gi, ks in GROUPS:
        L = 0 if band == 0 else 9
        if ks == [L]:
            continue
        for k in ks:
            out.append(chunks[k])
    return out


def kernel(feature: np.ndarray, label: np.ndarray) -> np.ndarray:
    global LAST_RESULTS
    if "nc" not in _CACHE:
        _CACHE["nc"] = _build_bass()
    nc = _CACHE["nc"]
    fean, in_maps = _prep_inputs(feature)
    res = run_bass_kernel_spmd(nc, in_maps, core_ids=list(range(NCORES)))
    LAST_RESULTS = res

    colsum = np.zeros(N, dtype=np.float64)
    for c in range(NCORES):
        r = res.results[c]
        rows = r["rows_out"].astype(np.float64).sum(axis=2)   # [P, 8, NGI] -> [P, 8]
        mir = r["mir_out"].reshape(NMIR, CW).astype(np.float64)
        A, B = c, 15 - c
        for band, g in ((0, A), (1, B)):
            for jt in range(NJT):
                colsum[g * CW + jt * P: g * CW + (jt + 1) * P] += rows[:, band * 4 + jt]
        for idx, g2 in enumerate(_mirror_chunks(c)):
            colsum[g2 * CW:(g2 + 1) * CW] += mir[idx]

    lab = np.asarray(label)
    counts = np.bincount(lab, minlength=int(lab.max()) + 1)
    order = np.argsort(lab, kind="stable")
    sorted_lab = lab[order]
    starts = np.concatenate(([0], np.nonzero(np.diff(sorted_lab))[0] + 1))
    S = np.zeros((counts.size, D), dtype=np.float32)
    S[sorted_lab[starts]] = np.add.reduceat(fean[order], starts, axis=0)
    possum = np.einsum("ij,ij->i", fean, S[lab]).astype(np.float64)

    loss_j = np.log(colsum) - possum / (TAL * counts[lab])
    return np.float32(loss_j.sum() / N)
